# revision 23
# baseline (speedup 1.0000x reference)
"""Grouped categorical log-softmax (segment logsumexp) on 8 Trainium2 cores.

Strategy: the index is sorted, so each segment is a contiguous run. On the host
we bucket segments by length (even canonical lengths 2,4,..,24 and a coarser
even tail, padding inside a slot with -80 so exp() contributes nothing),
shard every bucket evenly across the 8 cores, and lay each core's data out as a
dense [128, W_total] matrix where every bucket occupies a contiguous block of
columns holding 128*q fixed-length segment slots.

I/O is fp16: with standard-normal logits the outputs are O(10) and the 2e-2
relative-error budget dwarfs fp16 rounding (~5e-3 absolute), while the HBM
traffic — the roofline for this kernel — halves versus fp32.

Device pipeline per ~3k-column group: load x (sync HWDGE ring) -> exp (ScalarE)
-> fold halves of each slot with one packed fp16 add (DVE 2x mode) -> per-slot
reduce_sum (DVE) -> ln written twice with strided outputs (ScalarE), so the
subtract's broadcast operand is pair-duplicated -> subtract the per-slot ct
from x in place, expressed over [slot, L/2, 2] so every operand has a packed
16-bit last dim and DVE runs in 2x mode -> store x (sync ring). Groups are
staggered (subtract lags two groups) so all engines stream concurrently and
load/store DMA packets interleave on the 16 DMA engines for the whole kernel.
A pre-placed InstLoadActFuncSet pins the one activation table that holds BOTH
exp and ln (natural_log_exp_and_others), so alternating Exp/Ln costs no table
reloads. GPSIMD is deliberately left idle: measured on TRN2, its SBUF traffic
halves the DVE 2x throughput via port contention. out = x - log(sum(exp(x)))
is mathematically identical to the reference's max-normalized form; fp32/fp16
exp of standard-normal logits cannot overflow (sums stay <= 128*e^6 << 65504).

Length-1 segments are exactly 0 in the reference, so they are filled on the
host. Empty segments produce no output elements.
"""
from contextlib import ExitStack

import numpy as np

N_CORES = 8
P = 128
PAD_VAL = -80.0

# canonical slot lengths: even so each slot splits into packed fp16 pairs;
# coarse tail (>24 is <2% of segments) keeps the bucket/slice count low
_CANON_BASE = list(range(2, 26, 2)) + [28, 40, 56, 80, 128]


def _canon_lengths(max_len):
    canon = list(_CANON_BASE)
    while canon[-1] < max_len:
        canon.append(canon[-1] * 2)
    return np.asarray(canon, dtype=np.int64)


def _plan_buckets(index, num_segments):
    """Placement plan: maps every element to (core, flat offset) in the padded
    per-core [128, W_total] layout."""
    S = int(num_segments)
    idx = np.asarray(index).astype(np.int64)
    L = np.bincount(idx, minlength=S)
    starts = np.zeros(S + 1, dtype=np.int64)
    np.cumsum(L, out=starts[1:])

    seg1 = np.where(L == 1)[0]
    sel = np.where(L >= 2)[0]
    plan = dict(seg1=seg1, starts=starts)
    if len(sel) == 0:
        plan.update(W_total=0, buckets=[], e_src=np.empty(0, np.int64),
                    e_coreflat=np.empty(0, np.int64))
        return plan
    Ls = L[sel]
    canon = _canon_lengths(int(Ls.max()))
    Lc = canon[np.searchsorted(canon, Ls, side="left")]

    order = np.argsort(Lc, kind="stable")
    segs_sorted = sel[order]
    Ls_sorted = Ls[order]
    Lc_sorted = Lc[order]

    uniq, ustart, ucount = np.unique(Lc_sorted, return_index=True, return_counts=True)

    buckets = []                               # (Lb, q_b, col_b)
    col = 0
    nseg = len(segs_sorted)
    seg_core = np.empty(nseg, dtype=np.int64)
    seg_col = np.empty(nseg, dtype=np.int64)
    seg_prow = np.empty(nseg, dtype=np.int64)
    for Lb, s0, n in zip(uniq, ustart, ucount):
        Lb = int(Lb); s0 = int(s0); n = int(n)
        c = -(-n // N_CORES)                   # segs per core (ceil)
        q = -(-c // P)                         # slots per partition
        j = np.arange(n)
        core = j // c
        j_loc = j - core * c
        p = j_loc // q
        t = j_loc - p * q
        seg_core[s0:s0 + n] = core
        seg_prow[s0:s0 + n] = p
        seg_col[s0:s0 + n] = col + t * Lb
        buckets.append((Lb, q, col))
        col += q * Lb
    W_total = col

    tot_el = int(Ls_sorted.sum())
    off = np.zeros(nseg + 1, dtype=np.int64)
    np.cumsum(Ls_sorted, out=off[1:])
    within = np.arange(tot_el) - np.repeat(off[:-1], Ls_sorted)
    e_src = np.repeat(starts[segs_sorted], Ls_sorted) + within
    flat = seg_prow * W_total + seg_col
    e_flat = np.repeat(flat, Ls_sorted) + within
    e_core = np.repeat(seg_core, Ls_sorted)
    plan.update(W_total=W_total, buckets=buckets, e_src=e_src,
                e_coreflat=e_core * (P * W_total) + e_flat)
    return plan


def _build_inputs(logits, plan):
    W_total = plan["W_total"]
    xin = np.full(N_CORES * P * W_total, PAD_VAL, dtype=np.float16)
    xin[plan["e_coreflat"]] = np.asarray(logits, dtype=np.float32)[plan["e_src"]].astype(np.float16)
    return xin.reshape(N_CORES, P * W_total)


def _gather_output(results_flat, plan, n):
    out = np.zeros(n, dtype=np.float32)
    out[plan["e_src"]] = results_flat.reshape(-1)[plan["e_coreflat"]].astype(np.float32)
    out[plan["starts"][plan["seg1"]]] = 0.0
    return out


def _make_groups(buckets, target=3456, cap=4160, head=512, tail=768):
    """Split bucket column ranges into contiguous ~target-column groups of
    whole segment slots; each group is a list of (col, q_slice, Lb). The
    first/last groups are split small so the pipeline fills and drains
    quickly (fill = first load+exp, drain = last ln+sub+store)."""
    slices = []
    for (Lb, q, col) in buckets:
        qk = max(1, target // Lb)
        t = 0
        while t < q:
            qs = min(qk, q - t)
            slices.append((col + t * Lb, qs, Lb))
            t += qs
    groups, cur, cur_cols = [], [], 0
    for s in slices:
        scols = s[1] * s[2]
        if cur and cur_cols + scols > cap:
            groups.append(cur)
            cur, cur_cols = [], 0
        cur.append(s)
        cur_cols += scols
    if cur:
        groups.append(cur)

    def split_head(g, want):
        """Split group g into (head, rest) with head ~want columns."""
        h, rest, acc = [], [], 0
        for (col, qs, Lb) in g:
            if acc >= want:
                rest.append((col, qs, Lb))
                continue
            qtake = max(1, min(qs, (want - acc + Lb - 1) // Lb))
            h.append((col, qtake, Lb))
            acc += qtake * Lb
            if qtake < qs:
                rest.append((col + qtake * Lb, qs - qtake, Lb))
        return h, rest

    if len(groups) >= 2 and sum(qs * Lb for (_, qs, Lb) in groups[0]) > 2 * head:
        h, rest = split_head(groups[0], head)
        groups = [h, rest] + groups[1:]
    last_cols = sum(qs * Lb for (_, qs, Lb) in groups[-1])
    if len(groups) >= 2 and last_cols > 2 * tail:
        h, rest = split_head(groups[-1], last_cols - tail)
        groups = groups[:-1] + [h, rest]
    return groups


def _build_program(W_total, buckets, ebufs=4, target=3456, cap=4160):
    """Per-group software pipeline; see module docstring for the dataflow."""
    import concourse.bacc as bacc
    import concourse.mybir as mybir
    from concourse import tile

    F16 = mybir.dt.float16
    nc = bacc.Bacc("TRN2", target_bir_lowering=False, debug=False,
                   num_devices=N_CORES)
    xin = nc.dram_tensor("xin", [P * W_total], F16, kind="ExternalInput").ap()
    xout = nc.dram_tensor("xout", [P * W_total], F16, kind="ExternalOutput").ap()
    xin2d = xin.rearrange("(p w) -> p w", p=P)
    xout2d = xout.rearrange("(p w) -> p w", p=P)

    groups = _make_groups(buckets, target=target, cap=cap)
    n = len(groups)
    gspan = [(g[0][0], g[-1][0] + g[-1][1] * g[-1][2]) for g in groups]

    xts, ets, fts, sts, ct2s = {}, {}, {}, {}, {}

    with tile.TileContext(nc) as tc, ExitStack() as ctx:
        xpool = ctx.enter_context(tc.tile_pool(name="x", bufs=1))
        epool = ctx.enter_context(tc.tile_pool(name="e", bufs=ebufs))
        spool = ctx.enter_context(tc.tile_pool(name="s", bufs=1))

        # pin the activation table that serves BOTH Exp and Ln so the
        # compiler's table pass never inserts per-activation reloads
        nc.scalar.add_instruction(mybir.InstLoadActFuncSet(
            name="preload_act_exp_ln", act_func_set_id=6, ins=[], outs=[]))

        for gi in range(n):
            g0, g1 = gspan[gi]
            xt = xpool.tile([P, g1 - g0], F16, tag=f"x{gi}")
            nc.sync.dma_start(xt[:], xin2d[:, g0:g1])
            xts[gi] = xt

        def do_exp(gi):
            g0, g1 = gspan[gi]
            et = epool.tile([P, g1 - g0], F16, tag="e")
            nc.scalar.activation(et[:], xts[gi][:],
                                 mybir.ActivationFunctionType.Exp)
            ets[gi] = et

        FOLD_MIN = 768  # below this, one direct reduce beats fold+reduce

        def do_fold(gi):
            # ft[:, q, h] = et[:, q, h] + et[:, q, H+h]  (packed fp16 -> 2x);
            # small slices skip the fold (fixed instruction cost dominates)
            g0, _ = gspan[gi]
            wh = sum(qs * (Lb // 2) for (_, qs, Lb) in groups[gi])
            ft = epool.tile([P, wh], F16, tag="f")
            hoff = 0
            for (col, qs, Lb) in groups[gi]:
                c0 = col - g0
                H = Lb // 2
                if qs * Lb >= FOLD_MIN:
                    e3 = ets[gi][:, c0:c0 + qs * Lb].rearrange(
                        "p (q j h) -> p q j h", q=qs, j=2)
                    nc.vector.tensor_add(
                        ft[:, hoff:hoff + qs * H].rearrange(
                            "p (q h) -> p q h", q=qs),
                        e3[:, :, 0, :], e3[:, :, 1, :])
                hoff += qs * H
            fts[gi] = ft

        def do_reduce(gi):
            g0, _ = gspan[gi]
            qg = sum(qs for (_, qs, _) in groups[gi])
            st = spool.tile([P, qg], F16, tag=f"s{gi}")
            qoff = 0
            hoff = 0
            with nc.allow_low_precision("fp16 sum of <=64 fp16 pair-sums; "
                                        "abs err ~1e-3 vs 2e-2 gate"):
                for (col, qs, Lb) in groups[gi]:
                    H = Lb // 2
                    if qs * Lb >= FOLD_MIN:
                        nc.vector.reduce_sum(
                            st[:, qoff:qoff + qs],
                            fts[gi][:, hoff:hoff + qs * H].rearrange(
                                "p (q h) -> p q h", q=qs),
                            axis=mybir.AxisListType.X)
                    else:
                        c0 = col - g0
                        nc.vector.reduce_sum(
                            st[:, qoff:qoff + qs],
                            ets[gi][:, c0:c0 + qs * Lb].rearrange(
                                "p (q l) -> p q l", q=qs),
                            axis=mybir.AxisListType.X)
                    qoff += qs
                    hoff += qs * H
            sts[gi] = st

        def do_ln(gi):
            # two strided-output Ln's write ct pair-duplicated, so the
            # subtract's broadcast operand has a packed 16-bit last dim
            # (enables DVE 2x mode) with no extra copy on any engine
            qg = sum(qs for (_, qs, _) in groups[gi])
            ct2 = spool.tile([P, 2 * qg], F16, tag=f"d{gi}")
            ct2v = ct2[:].rearrange("p (q j) -> p q j", q=qg)
            nc.scalar.activation(ct2v[:, :, 0], sts[gi][:],
                                 mybir.ActivationFunctionType.Ln)
            nc.scalar.activation(ct2v[:, :, 1], sts[gi][:],
                                 mybir.ActivationFunctionType.Ln)
            ct2s[gi] = ct2

        def do_sub(gi):
            g0, _ = gspan[gi]
            xt = xts[gi]
            qoff = 0
            for (col, qs, Lb) in groups[gi]:
                c0 = col - g0
                H = Lb // 2
                x4 = xt[:, c0:c0 + qs * Lb].rearrange(
                    "p (q h j) -> p q h j", q=qs, h=H)
                c4 = ct2s[gi][:, 2 * qoff:2 * (qoff + qs)].rearrange(
                    "p (q j) -> p q j", q=qs).unsqueeze(2).broadcast_to(
                        [P, qs, H, 2])
                nc.vector.tensor_sub(x4, x4, c4)
                qoff += qs

        def do_store(gi):
            g0, g1 = gspan[gi]
            nc.sync.dma_start(xout2d[:, g0:g1], xts[gi][:])

        for gi in range(n):
            do_exp(gi)
            do_fold(gi)
            do_reduce(gi)
            if gi >= 1:
                do_ln(gi - 1)
            if gi == 1:
                # the small head group goes lag-1 so the store stream starts
                # early and interleaves with the remaining loads
                do_sub(0)
                do_store(0)
            elif gi >= 3:
                do_sub(gi - 2)
                do_store(gi - 2)
        if n >= 1:
            do_ln(n - 1)
        tail0 = max(1, n - 2) if n >= 2 else 0
        for gi in range(tail0, n):
            do_sub(gi)
            do_store(gi)
    nc.compile()
    return nc


_cache = {}


def _get_program(plan):
    key = (plan["W_total"], tuple(plan["buckets"]))
    if key not in _cache:
        _cache[key] = _build_program(plan["W_total"], plan["buckets"])
    return _cache[key]


def run_on_device(nc, xin_cores, trace=False, **kw):
    from concourse.bass_utils import run_bass_kernel_spmd
    in_maps = [{"xin": xin_cores[c]} for c in range(N_CORES)]
    res = run_bass_kernel_spmd(nc, in_maps, core_ids=list(range(N_CORES)),
                               trace=trace, **kw)
    out = np.stack([res.results[c]["xout"] for c in range(N_CORES)])
    return out, res


def kernel(logits, index, num_segments):
    logits = np.asarray(logits)
    n = logits.shape[0]
    plan = _plan_buckets(index, num_segments)
    if plan["W_total"] == 0:
        out = np.zeros(n, dtype=np.float32)
        out[plan["starts"][plan["seg1"]]] = 0.0
        return out
    xin = _build_inputs(logits, plan)
    nc = _get_program(plan)
    out_flat, _ = run_on_device(nc, xin)
    return _gather_output(out_flat, plan, n)


# revision 24
# speedup vs baseline: 1.1759x; 1.1759x over previous
"""Grouped categorical log-softmax (segment logsumexp) on 8 Trainium2 cores.

Strategy: the index is sorted, so each segment is a contiguous run. On the host
we bucket segments by length (even canonical lengths 2,4,..,24 and a coarser
even tail, padding inside a slot with -80 so exp() contributes nothing),
shard every bucket evenly across the 8 cores, and lay each core's data out as a
dense [128, W_total] matrix where every bucket occupies a contiguous block of
columns holding 128*q fixed-length segment slots.

I/O is fp16: with standard-normal logits the outputs are O(10) and the 2e-2
relative-error budget dwarfs fp16 rounding (~5e-3 absolute), while the HBM
traffic — the roofline for this kernel — halves versus fp32.

Device pipeline per ~3k-column group: load x (sync HWDGE ring) -> exp (ScalarE)
-> fold halves of each slot with one packed fp16 add (DVE 2x mode) -> per-slot
reduce_sum (DVE) -> ln written twice with strided outputs (ScalarE), so the
subtract's broadcast operand is pair-duplicated -> subtract the per-slot ct
from x in place, expressed over [slot, L/2, 2] so every operand has a packed
16-bit last dim and DVE runs in 2x mode -> store x (sync ring). Groups are
staggered (subtract lags two groups) so all engines stream concurrently and
load/store DMA packets interleave on the 16 DMA engines for the whole kernel.
A pre-placed InstLoadActFuncSet pins the one activation table that holds BOTH
exp and ln (natural_log_exp_and_others), so alternating Exp/Ln costs no table
reloads. GPSIMD is deliberately left idle: measured on TRN2, its SBUF traffic
halves the DVE 2x throughput via port contention. out = x - log(sum(exp(x)))
is mathematically identical to the reference's max-normalized form; fp32/fp16
exp of standard-normal logits cannot overflow (sums stay <= 128*e^6 << 65504).

Length-1 segments are exactly 0 in the reference, so they are filled on the
host. Empty segments produce no output elements.
"""
from contextlib import ExitStack

import numpy as np

N_CORES = 8
P = 128
PAD_VAL = -80.0

# canonical slot lengths: even so each slot splits into packed fp16 pairs;
# coarse tail (>24 is <2% of segments) keeps the bucket/slice count low
_CANON_BASE = list(range(2, 26, 2)) + [28, 40, 56, 80, 128]


def _canon_lengths(max_len):
    canon = list(_CANON_BASE)
    while canon[-1] < max_len:
        canon.append(canon[-1] * 2)
    return np.asarray(canon, dtype=np.int64)


def _plan_buckets(index, num_segments):
    """Placement plan: maps every element to (core, flat offset) in the padded
    per-core [128, W_total] layout."""
    S = int(num_segments)
    idx = np.asarray(index).astype(np.int64)
    L = np.bincount(idx, minlength=S)
    starts = np.zeros(S + 1, dtype=np.int64)
    np.cumsum(L, out=starts[1:])

    seg1 = np.where(L == 1)[0]
    sel = np.where(L >= 2)[0]
    plan = dict(seg1=seg1, starts=starts)
    if len(sel) == 0:
        plan.update(W_total=0, buckets=[], e_src=np.empty(0, np.int64),
                    e_coreflat=np.empty(0, np.int64))
        return plan
    Ls = L[sel]
    canon = _canon_lengths(int(Ls.max()))
    Lc = canon[np.searchsorted(canon, Ls, side="left")]

    order = np.argsort(Lc, kind="stable")
    segs_sorted = sel[order]
    Ls_sorted = Ls[order]
    Lc_sorted = Lc[order]

    uniq, ustart, ucount = np.unique(Lc_sorted, return_index=True, return_counts=True)

    buckets = []                               # (Lb, q_b, col_b)
    col = 0
    nseg = len(segs_sorted)
    seg_core = np.empty(nseg, dtype=np.int64)
    seg_col = np.empty(nseg, dtype=np.int64)
    seg_prow = np.empty(nseg, dtype=np.int64)
    for Lb, s0, n in zip(uniq, ustart, ucount):
        Lb = int(Lb); s0 = int(s0); n = int(n)
        c = -(-n // N_CORES)                   # segs per core (ceil)
        q = -(-c // P)                         # slots per partition
        j = np.arange(n)
        core = j // c
        j_loc = j - core * c
        p = j_loc // q
        t = j_loc - p * q
        seg_core[s0:s0 + n] = core
        seg_prow[s0:s0 + n] = p
        seg_col[s0:s0 + n] = col + t * Lb
        buckets.append((Lb, q, col))
        col += q * Lb
    W_total = col

    tot_el = int(Ls_sorted.sum())
    off = np.zeros(nseg + 1, dtype=np.int64)
    np.cumsum(Ls_sorted, out=off[1:])
    within = np.arange(tot_el) - np.repeat(off[:-1], Ls_sorted)
    e_src = np.repeat(starts[segs_sorted], Ls_sorted) + within
    flat = seg_prow * W_total + seg_col
    e_flat = np.repeat(flat, Ls_sorted) + within
    e_core = np.repeat(seg_core, Ls_sorted)
    plan.update(W_total=W_total, buckets=buckets, e_src=e_src,
                e_coreflat=e_core * (P * W_total) + e_flat)
    return plan


def _build_inputs(logits, plan):
    W_total = plan["W_total"]
    xin = np.full(N_CORES * P * W_total, PAD_VAL, dtype=np.float16)
    xin[plan["e_coreflat"]] = np.asarray(logits, dtype=np.float32)[plan["e_src"]].astype(np.float16)
    return xin.reshape(N_CORES, P * W_total)


def _gather_output(results_flat, plan, n):
    out = np.zeros(n, dtype=np.float32)
    out[plan["e_src"]] = results_flat.reshape(-1)[plan["e_coreflat"]].astype(np.float32)
    out[plan["starts"][plan["seg1"]]] = 0.0
    return out


def _make_groups(buckets, target=3456, cap=4160, head=512, tail=768):
    """Split bucket column ranges into contiguous ~target-column groups of
    whole segment slots; each group is a list of (col, q_slice, Lb). The
    first/last groups are split small so the pipeline fills and drains
    quickly (fill = first load+exp, drain = last ln+sub+store)."""
    slices = []
    for (Lb, q, col) in buckets:
        qk = max(1, target // Lb)
        t = 0
        while t < q:
            qs = min(qk, q - t)
            slices.append((col + t * Lb, qs, Lb))
            t += qs
    groups, cur, cur_cols = [], [], 0
    for s in slices:
        scols = s[1] * s[2]
        if cur and cur_cols + scols > cap:
            groups.append(cur)
            cur, cur_cols = [], 0
        cur.append(s)
        cur_cols += scols
    if cur:
        groups.append(cur)

    def split_head(g, want):
        """Split group g into (head, rest) with head ~want columns."""
        h, rest, acc = [], [], 0
        for (col, qs, Lb) in g:
            if acc >= want:
                rest.append((col, qs, Lb))
                continue
            qtake = max(1, min(qs, (want - acc + Lb - 1) // Lb))
            h.append((col, qtake, Lb))
            acc += qtake * Lb
            if qtake < qs:
                rest.append((col + qtake * Lb, qs - qtake, Lb))
        return h, rest

    if len(groups) >= 2 and sum(qs * Lb for (_, qs, Lb) in groups[0]) > 2 * head:
        h, rest = split_head(groups[0], head)
        groups = [h, rest] + groups[1:]
    last_cols = sum(qs * Lb for (_, qs, Lb) in groups[-1])
    if len(groups) >= 2 and last_cols > 2 * tail:
        h, rest = split_head(groups[-1], last_cols - tail)
        groups = groups[:-1] + [h, rest]
    return groups


def _build_program(W_total, buckets, ebufs=3, target=3456, cap=4160):
    """Per-group software pipeline; see module docstring for the dataflow."""
    import concourse.bacc as bacc
    import concourse.mybir as mybir
    from concourse import tile

    F16 = mybir.dt.float16
    nc = bacc.Bacc("TRN2", target_bir_lowering=False, debug=False,
                   num_devices=N_CORES)
    xin = nc.dram_tensor("xin", [P * W_total], F16, kind="ExternalInput").ap()
    xout = nc.dram_tensor("xout", [P * W_total], F16, kind="ExternalOutput").ap()
    xin2d = xin.rearrange("(p w) -> p w", p=P)
    xout2d = xout.rearrange("(p w) -> p w", p=P)

    groups = _make_groups(buckets, target=target, cap=cap)
    n = len(groups)
    gspan = [(g[0][0], g[-1][0] + g[-1][1] * g[-1][2]) for g in groups]

    xts, ets, fts, sts, ct2s = {}, {}, {}, {}, {}

    with tile.TileContext(nc) as tc, ExitStack() as ctx:
        xpool = ctx.enter_context(tc.tile_pool(name="x", bufs=1))
        epool = ctx.enter_context(tc.tile_pool(name="e", bufs=ebufs))
        spool = ctx.enter_context(tc.tile_pool(name="s", bufs=1))

        # pin the activation table that serves BOTH Exp and Ln so the
        # compiler's table pass never inserts per-activation reloads
        nc.scalar.add_instruction(mybir.InstLoadActFuncSet(
            name="preload_act_exp_ln", act_func_set_id=6, ins=[], outs=[]))

        for gi in range(n):
            g0, g1 = gspan[gi]
            xt = xpool.tile([P, g1 - g0], F16, tag=f"x{gi}")
            nc.sync.dma_start(xt[:], xin2d[:, g0:g1])
            xts[gi] = xt

        def do_exp(gi):
            g0, g1 = gspan[gi]
            et = epool.tile([P, g1 - g0], F16, tag="e")
            nc.scalar.activation(et[:], xts[gi][:],
                                 mybir.ActivationFunctionType.Exp)
            ets[gi] = et

        FOLD_MIN = 640  # below this, one direct reduce beats fold+reduce

        def do_fold(gi):
            # ft[:, q, h] = et[:, q, h] + et[:, q, H+h]  (packed fp16 -> 2x);
            # small slices skip the fold (fixed instruction cost dominates)
            g0, _ = gspan[gi]
            wh = sum(qs * (Lb // 2) for (_, qs, Lb) in groups[gi])
            ft = epool.tile([P, wh], F16, tag="f")
            hoff = 0
            for (col, qs, Lb) in groups[gi]:
                c0 = col - g0
                H = Lb // 2
                if qs * Lb >= FOLD_MIN:
                    e3 = ets[gi][:, c0:c0 + qs * Lb].rearrange(
                        "p (q j h) -> p q j h", q=qs, j=2)
                    nc.vector.tensor_add(
                        ft[:, hoff:hoff + qs * H].rearrange(
                            "p (q h) -> p q h", q=qs),
                        e3[:, :, 0, :], e3[:, :, 1, :])
                hoff += qs * H
            fts[gi] = ft

        def do_reduce(gi):
            g0, _ = gspan[gi]
            qg = sum(qs for (_, qs, _) in groups[gi])
            st = spool.tile([P, qg], F16, tag=f"s{gi}")
            qoff = 0
            hoff = 0
            with nc.allow_low_precision("fp16 sum of <=64 fp16 pair-sums; "
                                        "abs err ~1e-3 vs 2e-2 gate"):
                for (col, qs, Lb) in groups[gi]:
                    H = Lb // 2
                    if qs * Lb >= FOLD_MIN:
                        nc.vector.reduce_sum(
                            st[:, qoff:qoff + qs],
                            fts[gi][:, hoff:hoff + qs * H].rearrange(
                                "p (q h) -> p q h", q=qs),
                            axis=mybir.AxisListType.X)
                    else:
                        c0 = col - g0
                        nc.vector.reduce_sum(
                            st[:, qoff:qoff + qs],
                            ets[gi][:, c0:c0 + qs * Lb].rearrange(
                                "p (q l) -> p q l", q=qs),
                            axis=mybir.AxisListType.X)
                    qoff += qs
                    hoff += qs * H
            sts[gi] = st

        def do_ln(gi):
            # two strided-output Ln's write ct pair-duplicated, so the
            # subtract's broadcast operand has a packed 16-bit last dim
            # (enables DVE 2x mode) with no extra copy on any engine
            qg = sum(qs for (_, qs, _) in groups[gi])
            ct2 = spool.tile([P, 2 * qg], F16, tag=f"d{gi}")
            ct2v = ct2[:].rearrange("p (q j) -> p q j", q=qg)
            nc.scalar.activation(ct2v[:, :, 0], sts[gi][:],
                                 mybir.ActivationFunctionType.Ln)
            nc.scalar.activation(ct2v[:, :, 1], sts[gi][:],
                                 mybir.ActivationFunctionType.Ln)
            ct2s[gi] = ct2

        def do_sub(gi):
            g0, _ = gspan[gi]
            xt = xts[gi]
            qoff = 0
            for (col, qs, Lb) in groups[gi]:
                c0 = col - g0
                H = Lb // 2
                x4 = xt[:, c0:c0 + qs * Lb].rearrange(
                    "p (q h j) -> p q h j", q=qs, h=H)
                c4 = ct2s[gi][:, 2 * qoff:2 * (qoff + qs)].rearrange(
                    "p (q j) -> p q j", q=qs).unsqueeze(2).broadcast_to(
                        [P, qs, H, 2])
                nc.vector.tensor_sub(x4, x4, c4)
                qoff += qs

        def do_store(gi):
            g0, g1 = gspan[gi]
            nc.sync.dma_start(xout2d[:, g0:g1], xts[gi][:])

        for gi in range(n):
            do_exp(gi)
            do_fold(gi)
            do_reduce(gi)
            if gi >= 1:
                do_ln(gi - 1)
            if gi >= 2:
                do_sub(gi - 2)
                do_store(gi - 2)
        if n >= 1:
            do_ln(n - 1)
        for gi in range(max(0, n - 2), n):
            do_sub(gi)
            do_store(gi)
    nc.compile()
    return nc


_cache = {}


def _get_program(plan):
    key = (plan["W_total"], tuple(plan["buckets"]))
    if key not in _cache:
        _cache[key] = _build_program(plan["W_total"], plan["buckets"])
    return _cache[key]


def run_on_device(nc, xin_cores, trace=False, **kw):
    from concourse.bass_utils import run_bass_kernel_spmd
    in_maps = [{"xin": xin_cores[c]} for c in range(N_CORES)]
    res = run_bass_kernel_spmd(nc, in_maps, core_ids=list(range(N_CORES)),
                               trace=trace, **kw)
    out = np.stack([res.results[c]["xout"] for c in range(N_CORES)])
    return out, res


def kernel(logits, index, num_segments):
    logits = np.asarray(logits)
    n = logits.shape[0]
    plan = _plan_buckets(index, num_segments)
    if plan["W_total"] == 0:
        out = np.zeros(n, dtype=np.float32)
        out[plan["starts"][plan["seg1"]]] = 0.0
        return out
    xin = _build_inputs(logits, plan)
    nc = _get_program(plan)
    out_flat, _ = run_on_device(nc, xin)
    return _gather_output(out_flat, plan, n)
